# revision 12
# baseline (speedup 1.0000x reference)
"""MinRNN Trainium2 Bass kernel — vocab-table formulation.

Problem: minLSTM-style recurrence over sentences.
  x = emb[sentence]                       [B,S,E]
  f = sigmoid(x@Wf + bf); i = sigmoid(x@Wi + bi); h~ = x@Wh + bh
  f_n = f/(f+i); g = (i/(f+i)) * h~
  h_t = f_n_t * h_{t-1} + g_t   (scan over S, only final h needed)
  out = sigmoid((h@W1 + b1)@W2 + b2)      [B,1]

Key idea: the gates depend only on the *vocab id* of each token, and
B*S = 65536 tokens draw from only V = 32000 vocab entries. So instead of
3 GEMMs over 65536 token rows (the data-parallel baseline), compute
fn/gg per *vocab entry* (32000 rows) — 2.05x fewer FLOPs — then gather
per-token values from the table for the scan.

Sharding: model-parallel over units. Each core owns a 128-unit slice of
U=1024 and computes the full-vocab gate table for its slice (zero
cross-core traffic), then scans ALL 64 batch rows for its units. The
tiny head reduction (z1 partial [64,64] per core) is summed on the host
during unsharding.

Per-core dataflow (UC=128 units, V=32000, E=1024):
  phase 1: stream embT chunks [128e, EB, 640v] bf16 from HBM; for each
    128-vocab block: 8 matmuls (lhsT=embT block, rhs=W[128e, 384]) ->
    psum [128v, f|i|h 384] f32; ACT sigmoid on f|i, ACT copy h~; DVE
    custom ops FN (fn=f/(f+i)) and GG (gg=(h~)*(1-fn)) write bf16
    directly into the SBUF table [128, 250, fn128|gg128].
  phase 2: 32x dma_gather (SBUF source, transpose) pull per-token
    columns [128u, fn|gg, 2048tok]; tensor_tensor_scan per batch row;
    last element -> h_all [128u, 64b].
  head: z1T_c = W1_c.T @ h_all -> [64, 64] partial, DMA out; host sums
    partials over cores and applies the final 64->1 layer + sigmoid.
"""

import sys

if "/opt/trn_rl_repo" not in sys.path:
    sys.path.insert(0, "/opt/trn_rl_repo")

import numpy as np
import ml_dtypes

import concourse.bass as bass
import concourse.bacc as bacc
import concourse.mybir as mybir
from concourse.bass import ts
from concourse.tile import TileContext
from concourse.bass_utils import run_bass_kernel_spmd

N_CORES = 8
B, S, E, U, V = 64, 1024, 1024, 1024, 32000

UC = U // N_CORES            # units per core = 128
EB0 = E // 128               # contraction blocks (no bias row) = 8
VCHUNK = 640                 # vocab entries per streamed emb chunk
VBLK = VCHUNK // 128         # vocab blocks per chunk = 5
NTOK = B * S                 # tokens scanned per core = 65536
GIDX = 2048                  # tokens per dma_gather
NG = NTOK // GIDX            # 32
ROWS_PER_G = GIDX // S       # batch rows per gather tile = 2

F32 = mybir.dt.float32
BF16 = mybir.dt.bfloat16
I16 = mybir.dt.int16
AF = mybir.ActivationFunctionType
ALU = mybir.AluOpType


def _register_dve_op(name, spec):
    """Register a custom DVE op at runtime (self-pinning its uops sha)."""
    from concourse import dve_ops
    from concourse.dve_spec import lower, _has_src1
    from concourse.dve_uop import DveOpSpec

    if name in dve_ops.CUSTOM_DVE_SPECS:
        for op in dve_ops.OPS:
            if op.name == name:
                return op
    dve_ops._SUB_OPCODE_FOR_NAME[name] = dve_ops._CUSTOM_DVE_ROW_BASE + len(
        dve_ops.OPS
    )
    shas = {}
    for ver in ("v3", "v4"):
        s = DveOpSpec(
            name=name,
            opcode=dve_ops.get_dve_sub_opcode(name),
            uops=lower(spec, ver=ver),
            rd1_en=_has_src1(spec),
        )
        shas[ver] = s.sha(ver)
    op = dve_ops.DveOp(name, spec, subdim=False, uops_sha=shas)
    dve_ops.OPS.append(op)
    dve_ops.CUSTOM_DVE_SPECS[name] = spec
    return op


def _make_gate_ops():
    """Two fused gate ops:

    MINRNN_FN: fn = f / (f + i) via BITWISE_NOT reciprocal seed + 1 Newton
      step (Chebyshev pair; ~1.7e-3 max rel err on den in (0,2)).
      in0=f, in1=i, s0/s1 = recip constants.
    MINRNN_GG: gg = (h_pre + bh) * (1 - fn).  in0=h_pre, in1=fn, s0=bh.
    """
    import numpy as np
    from concourse.dve_spec import AluOp, Bin, C0, C1, One, Spec, Src0, Src1

    _den = Src0 + Src1
    _nd = Bin(AluOp.BITWISE_NOT, _den, _den)
    _y0 = _nd * C0
    _y1 = _y0 * (C1 - _den * _y0)

    def _ref_fn(in0, in1, c0, c1, c2):
        in1 = np.asarray(in1).reshape(np.asarray(in0).shape)
        den = (in0 + in1).astype(np.float32)
        nd = (~den.view(np.int32)).view(np.float32)
        y0 = (nd * np.float32(c0)).astype(np.float32)
        y1 = (y0 * (np.float32(c1) - den * y0)).astype(np.float32)
        return (in0 * y1).astype(np.float32)

    fn_op = _register_dve_op(
        "MINRNN_FN", Spec(body=Src0 * _y1, reference=_ref_fn)
    )

    def _ref_gg(in0, in1, c0, c1, c2):
        c0 = np.asarray(c0, np.float32)
        in1 = np.asarray(in1).reshape(np.asarray(in0).shape)
        return ((in0 + c0) * (np.float32(1.0) - in1)).astype(np.float32)

    gg_op = _register_dve_op(
        "MINRNN_GG",
        Spec(body=(Src0 + C0) * (One - Src1), reference=_ref_gg),
    )
    return fn_op, gg_op


RECIP_C0 = -0.23549792
RECIP_C1 = 2.0017324


def build_nc(nchunk, with_bias_row=False):
    """Single-core SPMD program (unit-sharded; same program on all cores).

    nchunk: number of VCHUNK-sized vocab chunks actually used (the host
    dedups the vocab to the entries referenced by `sentence`).
    with_bias_row: append a 9th contraction block carrying the gate
    biases (emb extended with a ones-row on the host). Skipped when the
    biases are all zero (the reference initializes them to zero).
    """
    eb = EB0 + (1 if with_bias_row else 0)
    vranks = nchunk * VBLK

    nc = bacc.Bacc("TRN2", target_bir_lowering=False)
    FN_OP, GG_OP = _make_gate_ops()
    from concourse import library_config

    embt_t = nc.dram_tensor(
        "embt", [128, nchunk, eb, VCHUNK], BF16, kind="ExternalInput"
    )
    wall_t = nc.dram_tensor("wall", [128, eb, 3 * UC], BF16, kind="ExternalInput")
    idx_t = nc.dram_tensor("idx", [128, NTOK // 16], I16, kind="ExternalInput")
    w1_t = nc.dram_tensor("w1", [128, 64], F32, kind="ExternalInput")
    out_t = nc.dram_tensor("out", [64, B], F32, kind="ExternalOutput")

    with TileContext(nc) as tc:
        with (
            tc.tile_pool(name="singles", bufs=1) as singles,
            tc.tile_pool(name="emb", bufs=2) as embp,
            tc.tile_pool(name="sig", bufs=2) as sigp,
            tc.tile_pool(name="hst", bufs=2) as hstp,
            tc.tile_pool(name="gt", bufs=2) as gtp,
            tc.tile_pool(name="sc", bufs=2) as scp,
            tc.tile_pool(name="gates", bufs=7, space="PSUM") as gps,
            tc.tile_pool(name="headps", bufs=1, space="PSUM") as hps,
        ):
            # dma_gather ucode lives in the mlp gpsimd library.
            nc.gpsimd.load_library(library_config.mlp)
            # idx on the ACT ring so it doesn't queue behind weights.
            idx_sb = singles.tile([128, NTOK // 16], I16, tag="idx")
            nc.scalar.dma_start(out=idx_sb[:], in_=idx_t[:])
            wall_sb = singles.tile([128, eb, 3 * UC], BF16, tag="wall")
            nc.sync.dma_start(out=wall_sb[:], in_=wall_t[:])
            w1_sb = singles.tile([128, 64], F32, tag="w1")
            nc.sync.dma_start(out=w1_sb[:], in_=w1_t[:])

            # fn|gg per vocab entry: partition = v % 128, rank = v // 128.
            table = singles.tile([128, vranks, 2 * UC], BF16, tag="table")
            h_all = singles.tile([128, B], F32, tag="h_all")

            # --- phase 1: gate table over the used vocab ---
            for c in range(nchunk):
                et = embp.tile([128, eb, VCHUNK], BF16, tag="et")
                nc.sync.dma_start(out=et[:], in_=embt_t[:, c])
                sig = sigp.tile([128, VBLK, 2 * UC], BF16, tag="sig")
                hst = hstp.tile([128, VBLK, UC], BF16, tag="hst")
                for j in range(VBLK):
                    ps = gps.tile([128, 512], F32, tag="ps")
                    for m in range(eb):
                        nc.tensor.matmul(
                            ps[:, 0 : 3 * UC],
                            lhsT=et[:, m, ts(j, 128)],
                            rhs=wall_sb[:, m, :],
                            start=(m == 0),
                            stop=(m == eb - 1),
                        )
                    nc.scalar.activation(
                        sig[:, j, :], ps[:, 0 : 2 * UC], AF.Sigmoid
                    )
                    nc.scalar.activation(
                        hst[:, j, :], ps[:, 2 * UC : 3 * UC], AF.Copy
                    )
                r0 = c * VBLK
                nc.vector._custom_dve(
                    FN_OP,
                    out=table[:, r0 : r0 + VBLK, 0:UC],
                    in0=sig[:, :, 0:UC],
                    in1=sig[:, :, UC : 2 * UC],
                    s0=RECIP_C0,
                    s1=RECIP_C1,
                )
                nc.vector._custom_dve(
                    GG_OP,
                    out=table[:, r0 : r0 + VBLK, UC : 2 * UC],
                    in0=hst[:, :, :],
                    in1=table[:, r0 : r0 + VBLK, 0:UC],
                    s0=0.0,
                )

            # --- phase 2: per-token gather + scan ---
            for g in range(NG):
                gt = gtp.tile([128, 2, GIDX], BF16, tag="gt")
                nc.gpsimd.dma_gather(
                    gt[:],
                    table[:],
                    idx_sb[:, g * (GIDX // 16) : (g + 1) * (GIDX // 16)],
                    GIDX,
                    GIDX,
                    2 * UC,
                    transpose=True,
                    single_packet=False,
                    sbuf_tokens_per_rank=128,
                    sbuf_free_dim_per_rank=2 * UC * 2,
                )
                for j in range(ROWS_PER_G):
                    row = g * ROWS_PER_G + j
                    sc = scp.tile([128, S], F32, tag="sc")
                    nc.vector.tensor_tensor_scan(
                        out=sc[:],
                        data0=gt[:, 0, ts(j, S)],
                        data1=gt[:, 1, ts(j, S)],
                        initial=0.0,
                        op0=ALU.mult,
                        op1=ALU.add,
                    )
                    nc.vector.tensor_copy(
                        out=h_all[:, row : row + 1], in_=sc[:, S - 1 : S]
                    )

            # --- head partial: z1T_c = W1_c.T @ h_all  [64 out, 64 batch] ---
            z1p = hps.tile([64, B], F32, tag="z1p")
            nc.tensor.matmul(
                z1p[:], lhsT=w1_sb[:], rhs=h_all[:], start=True, stop=True
            )
            z1sb = singles.tile([64, B], F32, tag="z1sb")
            nc.vector.tensor_copy(out=z1sb[:], in_=z1p[:])
            nc.scalar.dma_start(out=out_t[:], in_=z1sb[:])

    nc.compile()
    return nc


def plan_shapes(sentence):
    """Dedup the vocab to the entries `sentence` references."""
    uniq, inv = np.unique(np.asarray(sentence).reshape(-1), return_inverse=True)
    nchunk = -(-uniq.size // VCHUNK)
    return uniq, inv, nchunk


def make_in_maps(sentence, emb, Wf, bf, Wi, bi, Wh, bh, W1, b1, W2, b2,
                 n_rows=None, n_cores=N_CORES):
    """Shard/repack full inputs into per-core input maps."""
    with_bias = bool(
        np.any(np.asarray(bf)) or np.any(np.asarray(bi)) or np.any(np.asarray(bh))
    )
    eb = EB0 + (1 if with_bias else 0)

    uniq, inv, nchunk = plan_shapes(sentence)
    nv = nchunk * VCHUNK

    # embT chunks over used vocab: embt[p, c, m, j] = emb_u[c*VCHUNK+j, m*128+p]
    emb_bf = np.zeros((nv, E + (128 if with_bias else 0)), ml_dtypes.bfloat16)
    emb_bf[: uniq.size, :E] = np.asarray(emb, dtype=np.float32)[uniq].astype(
        ml_dtypes.bfloat16
    )
    if with_bias:
        emb_bf[: uniq.size, E] = 1.0
    embt = np.ascontiguousarray(
        emb_bf.reshape(nchunk, VCHUNK, eb, 128).transpose(3, 0, 2, 1)
    )

    # idx: token i of gather g -> idx[i%16, g*(GIDX//16) + i//16].
    # The Q7 tx/rx cores each read idxs through their own 16-partition
    # window, so the values must be replicated across all 8 windows.
    tok = inv.reshape(-1).astype(np.int16)
    assert tok.size == NTOK
    idx = np.tile(
        np.ascontiguousarray(tok.reshape(NTOK // 16, 16).T), (8, 1)
    )

    in_maps = []
    for c in range(n_cores):
        u0 = c * UC
        wall = np.zeros((128, eb, 3 * UC), np.float32)
        for g_i, (W, bvec) in enumerate(
            ((Wf, bf), (Wi, bi), (Wh, bh))
        ):
            Wc = np.asarray(W, dtype=np.float32)[:, u0 : u0 + UC]
            wall[:, :EB0, g_i * UC : (g_i + 1) * UC] = Wc.reshape(
                EB0, 128, UC
            ).transpose(1, 0, 2)
            if with_bias:
                wall[0, EB0, g_i * UC : (g_i + 1) * UC] = np.asarray(
                    bvec, np.float32
                )[u0 : u0 + UC]
        in_maps.append(
            {
                "embt": embt,
                "wall": wall.astype(ml_dtypes.bfloat16),
                "idx": idx,
                "w1": np.ascontiguousarray(
                    np.asarray(W1, dtype=np.float32)[u0 : u0 + UC, :]
                ),
            }
        )
    return in_maps


_NC_CACHE = {}


def kernel(**inputs):
    sentence = np.asarray(inputs["sentence"])
    bf = np.asarray(inputs["bf"])
    bi = np.asarray(inputs["bi"])
    bh = np.asarray(inputs["bh"])
    with_bias = bool(np.any(bf) or np.any(bi) or np.any(bh))
    _, _, nchunk = plan_shapes(sentence)
    key = f"v{nchunk}" + ("_bias" if with_bias else "")
    if key not in _NC_CACHE:
        _NC_CACHE[key] = build_nc(nchunk, with_bias_row=with_bias)
    nc = _NC_CACHE[key]
    _NC_CACHE["full"] = nc  # alias for external harnesses (test.py)
    in_maps = make_in_maps(
        sentence,
        np.asarray(inputs["emb"]), np.asarray(inputs["Wf"]),
        bf, np.asarray(inputs["Wi"]), bi,
        np.asarray(inputs["Wh"]), bh,
        np.asarray(inputs["W1"]), np.asarray(inputs["b1"]),
        np.asarray(inputs["W2"]), np.asarray(inputs["b2"]),
    )
    res = run_bass_kernel_spmd(nc, in_maps, core_ids=list(range(N_CORES)))
    # z1T partials [64 out, 64 batch] per core; finish the tiny head here
    # (part of unsharding: sum over the unit shards).
    z1T = np.zeros((64, B), np.float64)
    for c in range(N_CORES):
        z1T += np.asarray(res.results[c]["out"], dtype=np.float64)
    z1 = z1T.T + np.asarray(inputs["b1"], np.float64)
    z2 = z1 @ np.asarray(inputs["W2"], np.float64) + np.asarray(
        inputs["b2"], np.float64
    )
    return (1.0 / (1.0 + np.exp(-z2))).astype(np.float32).reshape(B, 1)
